# revision 1
# baseline (speedup 1.0000x reference)
"""Trainium2 Bass kernel for nn_GatedShortConvBlock.

Full inputs -> full output. Data-parallel over batch: batch b -> core b.
Per-core dataflow (T=4096 tokens, D=1024, H=2048, K=4):
  LN stats on DVE (bn_stats over natural-layout x tiles); normalization
  applied in [d, t] layout to a host-pre-transposed xT via PE rank-1
  broadcast of the mean/var rows -> in-proj f32r matmuls (N=512) ->
  sigmoid gates on ACT (+bias fused) -> depthwise causal conv on DVE
  via scalar_tensor_tensor FMAs ([h, t] layout, per-partition weights)
  -> out-proj f32r matmuls -> +bias +residual -> out.
Weights are pre-transposed/reordered on the host so every DMA is a
large contiguous transfer and every matmul operand is layout-native.
"""

import sys
import types

import numpy as np

B, T, D, H = 8, 4096, 1024, 2048
KCONV = 4
N_CORES = 8
TB = 512            # tokens per t-block
NTB = T // TB       # 8
NSUB = TB // 128    # 4
KC = H // 128       # 16 channel blocks
DC = D // 128       # 8 d-chunks
EPS = 1e-5

_CACHE = {}


def _install_ntff_hook():
    if "antenv.axon_hooks" in sys.modules:
        return
    try:
        import trn_agent_boot.trn_boot as tb

        hook = tb._ntff_profile_via_ctypes("/opt/axon/libaxon_pjrt.so")
        mod = types.ModuleType("antenv.axon_hooks")
        mod.get_axon_ntff_profile_hook = lambda: hook
        mod.set_axon_ntff_profile_hook = lambda h: None
        sys.modules["antenv.axon_hooks"] = mod
    except Exception:
        pass


def _build_nc():
    import concourse.bass as bass  # noqa: F401
    import concourse.tile as tile
    from concourse import bacc, mybir

    f32 = mybir.dt.float32
    f32r = mybir.dt.float32r
    AF = mybir.ActivationFunctionType
    OP = mybir.AluOpType

    nc = bacc.Bacc("TRN2", target_bir_lowering=False, debug=False,
                   num_devices=N_CORES)
    x_d = nc.dram_tensor("x", [T, D], f32, kind="ExternalInput").ap()
    xt_d = nc.dram_tensor("xt_proc", [NTB, 128, DC, TB], f32,
                          kind="ExternalInput").ap()
    win_d = nc.dram_tensor("w_in_proc", [KC, 128, 3 * DC * 128], f32,
                           kind="ExternalInput").ap()
    d_d = nc.dram_tensor("d_proc", [128, KC, 3], f32,
                         kind="ExternalInput").ap()
    cw_d = nc.dram_tensor("cw_proc", [128, KC, KCONV], f32,
                          kind="ExternalInput").ap()
    cb_d = nc.dram_tensor("cb_proc", [128, KC], f32,
                          kind="ExternalInput").ap()
    wout_d = nc.dram_tensor("w_out_proc", [128, KC, D], f32,
                            kind="ExternalInput").ap()
    bout_d = nc.dram_tensor("bout_rep", [128, D], f32,
                            kind="ExternalInput").ap()
    iden_d = nc.dram_tensor("iden", [128, 128], f32,
                            kind="ExternalInput").ap()
    out_d = nc.dram_tensor("out", [T, D], f32, kind="ExternalOutput").ap()

    from contextlib import ExitStack

    with tile.TileContext(nc) as tc, ExitStack() as ctx:
        consts = ctx.enter_context(tc.tile_pool(name="consts", bufs=1))
        wgp = ctx.enter_context(tc.tile_pool(name="wg", bufs=2))
        xp = ctx.enter_context(tc.tile_pool(name="xp", bufs=2))
        lnp = ctx.enter_context(tc.tile_pool(name="ln", bufs=4))
        xtp = ctx.enter_context(tc.tile_pool(name="xtp", bufs=2))
        rowp = ctx.enter_context(tc.tile_pool(name="rowp", bufs=1))
        scp = ctx.enter_context(tc.tile_pool(name="scp", bufs=2))
        rsp = ctx.enter_context(tc.tile_pool(name="rsp", bufs=1))
        hid2p = ctx.enter_context(tc.tile_pool(name="hid2", bufs=1))
        sgp = ctx.enter_context(tc.tile_pool(name="sg", bufs=2))
        hbp = ctx.enter_context(tc.tile_pool(name="hb", bufs=2))
        accp = ctx.enter_context(tc.tile_pool(name="acc", bufs=2))
        outp = ctx.enter_context(tc.tile_pool(name="outp", bufs=2))
        xrp = ctx.enter_context(tc.tile_pool(name="xr", bufs=2))
        pst = ctx.enter_context(tc.tile_pool(name="pst", bufs=3, space="PSUM"))
        psm = ctx.enter_context(tc.tile_pool(name="psm", bufs=5, space="PSUM"))
        if True:
            wout_sb = consts.tile([128, KC, D], f32r)
            nc.sync.dma_start(out=wout_sb, in_=wout_d.bitcast(f32r))
            d_sb = consts.tile([128, KC, 3], f32)
            nc.sync.dma_start(out=d_sb, in_=d_d)
            cw_sb = consts.tile([128, KC, KCONV], f32)
            nc.sync.dma_start(out=cw_sb, in_=cw_d)
            cb_sb = consts.tile([128, KC], f32)
            nc.sync.dma_start(out=cb_sb, in_=cb_d)
            bout_sb = consts.tile([128, D], f32)
            nc.sync.dma_start(out=bout_sb, in_=bout_d)
            iden_sb = consts.tile([128, 128], f32)
            nc.sync.dma_start(out=iden_sb, in_=iden_d)
            eps_sb = consts.tile([128, 1], f32)
            nc.vector.memset(eps_sb, EPS)
            ones_sb = consts.tile([1, 128], f32)
            nc.vector.memset(ones_sb, 1.0)
            state = consts.tile([128, KC, 3], f32)
            nc.vector.memset(state, 0.0)

            for tb in range(NTB):
                t0 = tb * TB
                # ---- phase A: LN stats + broadcast-normalize xT ----
                xT = xtp.tile([128, DC, TB], f32, name="xT")
                nc.sync.dma_start(out=xT.bitcast(f32r),
                                  in_=xt_d[tb].bitcast(f32r))
                mv4 = lnp.tile([128, 2 * NSUB], f32, name="mv4")
                for s in range(NSUB):
                    xt = xp.tile([128, D], f32, name="xt")
                    nc.sync.dma_start(
                        out=xt, in_=x_d[t0 + s * 128:t0 + (s + 1) * 128, :])
                    stats = lnp.tile([128, 2, 6], f32, name="stats")
                    for g in range(2):
                        nc.vector.bn_stats(
                            out=stats[:, g, :], in_=xt[:, g * 512:(g + 1) * 512])
                    nc.vector.bn_aggr(
                        out=mv4[:, 2 * s:2 * s + 2], in_=stats)
                # transpose [128, 8] -> [8, 128]: rows f = 2s+c (mu, var)
                pt = pst.tile([128, TB], f32, name="bc")
                nc.tensor.transpose(pt[0:8, 0:128], mv4, iden_sb)
                mv8 = rowp.tile([8, 128], f32, name="mv8")
                nc.scalar.copy(out=mv8, in_=pt[0:8, 0:128])
                mrow = rowp.tile([1, 2 * NSUB, 128], f32, name="mrow")
                nc.sync.dma_start(out=mrow, in_=mv8)
                # rank-1 broadcasts: mu_b / var_b [128, TB]
                mu_b = pst.tile([128, TB], f32, name="bc")
                nc.tensor.matmul(mu_b, ones_sb, mrow[:, 0:2 * NSUB:2, :],
                                 start=True, stop=True)
                var_b = pst.tile([128, TB], f32, name="bc")
                nc.tensor.matmul(var_b, ones_sb, mrow[:, 1:2 * NSUB:2, :],
                                 start=True, stop=True)
                rstd_b = rsp.tile([128, TB], f32, name="rstd_b")
                nc.scalar.activation(out=rstd_b, in_=var_b, func=AF.Sqrt,
                                     bias=eps_sb, scale=1.0)
                nc.vector.reciprocal(out=rstd_b, in_=rstd_b)
                for j in range(DC):
                    sc = scp.tile([128, TB], f32, name="sc")
                    nc.vector.tensor_sub(out=sc, in0=xT[:, j, :], in1=mu_b)
                    nc.vector.tensor_mul(out=xT[:, j, :].bitcast(f32r),
                                         in0=sc, in1=rstd_b)

                # ---- phase B: in-proj + gates + conv per channel block ----
                hid2 = hid2p.tile([128, KC, TB], f32r, name="hid2")
                for k in range(KC):
                    wgt = wgp.tile([128, 3 * DC * 128], f32r, name="wgt")
                    nc.sync.dma_start(out=wgt, in_=win_d[k].bitcast(f32r))
                    ps3 = []
                    for t3 in range(3):
                        ps = psm.tile([128, TB], f32, name="mm")
                        for j in range(DC):
                            nc.tensor.matmul(
                                ps,
                                wgt[:, (t3 * DC + j) * 128:
                                    (t3 * DC + j + 1) * 128],
                                xT[:, j, :].bitcast(f32r),
                                start=(j == 0), stop=(j == DC - 1))
                        ps3.append(ps)
                    sigb = sgp.tile([128, TB], f32, name="sigb")
                    nc.scalar.activation(out=sigb, in_=ps3[0], func=AF.Sigmoid,
                                         bias=d_sb[:, k, 0:1], scale=1.0)
                    sigc = sgp.tile([128, TB], f32, name="sigc")
                    nc.scalar.activation(out=sigc, in_=ps3[1], func=AF.Sigmoid,
                                         bias=d_sb[:, k, 1:2], scale=1.0)
                    hidb = hbp.tile([128, TB + 3], f32, name="hidb")
                    nc.vector.tensor_copy(out=hidb[:, 0:3], in_=state[:, k, :])
                    nc.vector.scalar_tensor_tensor(
                        out=hidb[:, 3:TB + 3], in0=ps3[2],
                        scalar=d_sb[:, k, 2:3], in1=sigb,
                        op0=OP.add, op1=OP.mult)
                    nc.vector.tensor_copy(
                        out=state[:, k, :], in_=hidb[:, TB:TB + 3])
                    a0 = accp.tile([128, TB], f32, name="acc0")
                    a1 = accp.tile([128, TB], f32, name="acc1")
                    nc.vector.tensor_scalar_mul(
                        out=a0, in0=hidb[:, 0:TB], scalar1=cw_sb[:, k, 0:1])
                    nc.vector.scalar_tensor_tensor(
                        out=a1, in0=hidb[:, 1:TB + 1],
                        scalar=cw_sb[:, k, 1:2], in1=a0,
                        op0=OP.mult, op1=OP.add)
                    nc.vector.scalar_tensor_tensor(
                        out=a0, in0=hidb[:, 2:TB + 2],
                        scalar=cw_sb[:, k, 2:3], in1=a1,
                        op0=OP.mult, op1=OP.add)
                    nc.vector.scalar_tensor_tensor(
                        out=a1, in0=hidb[:, 3:TB + 3],
                        scalar=cw_sb[:, k, 3:4], in1=a0,
                        op0=OP.mult, op1=OP.add)
                    nc.vector.scalar_tensor_tensor(
                        out=hid2[:, k, :], in0=a1, scalar=cb_sb[:, k:k + 1],
                        in1=sigc, op0=OP.add, op1=OP.mult)

                # ---- phase C: out-proj + bias + residual ----
                for s in range(NSUB):
                    ps0 = psm.tile([128, 512], f32, name="mm")
                    ps1 = psm.tile([128, 512], f32, name="mm")
                    for j in range(KC):
                        nc.tensor.matmul(
                            ps0,
                            hid2[:, j, s * 128:(s + 1) * 128],
                            wout_sb[:, j, 0:512],
                            start=(j == 0), stop=(j == KC - 1))
                        nc.tensor.matmul(
                            ps1,
                            hid2[:, j, s * 128:(s + 1) * 128],
                            wout_sb[:, j, 512:1024],
                            start=(j == 0), stop=(j == KC - 1))
                    for dh, ps in ((0, ps0), (1, ps1)):
                        xr = xrp.tile([128, 512], f32, name="xr")
                        nc.sync.dma_start(
                            out=xr,
                            in_=x_d[t0 + s * 128:t0 + (s + 1) * 128,
                                    dh * 512:(dh + 1) * 512])
                        ot = outp.tile([128, 512], f32, name="ot")
                        nc.vector.tensor_add(
                            out=ot, in0=ps,
                            in1=bout_sb[:, dh * 512:(dh + 1) * 512])
                        nc.vector.tensor_add(out=ot, in0=ot, in1=xr)
                        nc.sync.dma_start(
                            out=out_d[t0 + s * 128:t0 + (s + 1) * 128,
                                      dh * 512:(dh + 1) * 512],
                            in_=ot)
    nc.compile()
    return nc


def _prep_inputs(x, ln_g, ln_b, w_in, b_in, conv_w, conv_b, w_out, b_out):
    x = np.asarray(x, np.float32)
    ln_g = np.asarray(ln_g, np.float32)
    ln_b = np.asarray(ln_b, np.float32)
    w_in = np.asarray(w_in, np.float32)
    b_in = np.asarray(b_in, np.float32)
    conv_w = np.asarray(conv_w, np.float32)
    conv_b = np.asarray(conv_b, np.float32)
    w_out = np.asarray(w_out, np.float32)
    b_out = np.asarray(b_out, np.float32)

    wg = w_in * ln_g[None, :]                       # [3H, D]
    dv = b_in + w_in @ ln_b                         # [3H]
    # [third, k, m, j, p] -> [k, p, third, j, m]
    wblk = wg.reshape(3, KC, 128, DC, 128)
    w_in_proc = np.ascontiguousarray(
        wblk.transpose(1, 4, 0, 3, 2)).reshape(KC, 128, 3 * DC * 128)
    d_proc = np.ascontiguousarray(dv.reshape(3, KC, 128).transpose(2, 1, 0))
    cw_proc = np.ascontiguousarray(
        conv_w.reshape(KC, 128, KCONV).transpose(1, 0, 2))
    cb_proc = np.ascontiguousarray(conv_b.reshape(KC, 128).T)
    w_out_proc = np.ascontiguousarray(
        w_out.T.reshape(KC, 128, D).transpose(1, 0, 2))
    bout_rep = np.ascontiguousarray(np.tile(b_out[None, :], (128, 1)))
    iden = np.eye(128, dtype=np.float32)

    shared = {
        "w_in_proc": w_in_proc, "d_proc": d_proc, "cw_proc": cw_proc,
        "cb_proc": cb_proc, "w_out_proc": w_out_proc, "bout_rep": bout_rep,
        "iden": iden,
    }
    maps = []
    for b in range(B):
        xb = np.ascontiguousarray(x[b])
        # xt_proc[tb, p, j, t] = x[b, 512*tb + t, 128*j + p]
        xt_proc = np.ascontiguousarray(
            xb.T.reshape(DC, 128, NTB, TB).transpose(2, 1, 0, 3))
        maps.append(dict(shared, x=xb, xt_proc=xt_proc))
    return maps


def run(inputs, trace=False):
    _install_ntff_hook()
    import concourse.bass_utils as bu

    bu.upload_artifacts = lambda d: "local://" + d
    if "nc" not in _CACHE:
        _CACHE["nc"] = _build_nc()
    nc = _CACHE["nc"]
    in_maps = _prep_inputs(**inputs)
    res = bu.run_bass_kernel_spmd(nc, in_maps, list(range(N_CORES)),
                                  trace=trace)
    out = np.stack([res.results[b]["out"] for b in range(B)])
    return out.astype(np.float32), res


def kernel(**inputs):
    out, _ = run(inputs, trace=False)
    return out



# revision 3
# speedup vs baseline: 1.1915x; 1.1915x over previous
"""Trainium2 Bass kernel for nn_GatedShortConvBlock.

Full inputs -> full output. Data-parallel over batch: batch b -> core b.
Per-core dataflow (T=4096 tokens, D=1024, H=2048, K=4), all-bf16 matmuls:
  w_in (bf16, 12.6MB) and w_out (bf16, 4MB) resident in SBUF, loaded once.
  LN stats on DVE (bn_stats over natural-layout x tiles) -> mean/rstd rows
  via PE transpose + tiny sqrt/recip -> rank-1 PE broadcast -> normalize
  pre-transposed bf16 xT in [d, t] layout -> in-proj bf16 matmuls (N=512)
  -> sigmoid gates + hidden bias on ACT (bf16 out) -> depthwise causal
  conv on DVE via bf16 scalar_tensor_tensor FMAs -> out-proj bf16 matmuls
  (j-outer, 2-bank groups so they pipeline behind the conv) -> +residual
  (host-fused x + b_out) -> out.
"""

import sys
import types

import numpy as np

B, T, D, H = 8, 4096, 1024, 2048
KCONV = 4
N_CORES = 8
TB = 512            # tokens per t-block
NTB = T // TB       # 8
NSUB = TB // 128    # 4
KC = H // 128       # 16 channel blocks
DC = D // 128       # 8 d-chunks
EPS = 1e-5

_CACHE = {}


def _install_ntff_hook():
    if "antenv.axon_hooks" in sys.modules:
        return
    try:
        import trn_agent_boot.trn_boot as tb

        hook = tb._ntff_profile_via_ctypes("/opt/axon/libaxon_pjrt.so")
        mod = types.ModuleType("antenv.axon_hooks")
        mod.get_axon_ntff_profile_hook = lambda: hook
        mod.set_axon_ntff_profile_hook = lambda h: None
        sys.modules["antenv.axon_hooks"] = mod
    except Exception:
        pass


def _build_nc():
    import concourse.bass as bass  # noqa: F401
    import concourse.tile as tile
    from concourse import bacc, mybir

    f32 = mybir.dt.float32
    bf16 = mybir.dt.bfloat16
    AF = mybir.ActivationFunctionType
    OP = mybir.AluOpType

    nc = bacc.Bacc("TRN2", target_bir_lowering=False, debug=False,
                   num_devices=N_CORES)
    x_d = nc.dram_tensor("x", [T, D], f32, kind="ExternalInput").ap()
    xr_d = nc.dram_tensor("xr_proc", [T, D], f32, kind="ExternalInput").ap()
    xt_d = nc.dram_tensor("xt_proc", [NTB, 128, DC, TB], bf16,
                          kind="ExternalInput").ap()
    win_d = nc.dram_tensor("w_in_proc", [KC, 128, 3 * DC * 128], bf16,
                           kind="ExternalInput").ap()
    d_d = nc.dram_tensor("d_proc", [128, KC, 3], f32,
                         kind="ExternalInput").ap()
    cw_d = nc.dram_tensor("cw_proc", [128, KC, KCONV], f32,
                          kind="ExternalInput").ap()
    cb_d = nc.dram_tensor("cb_proc", [128, KC], f32,
                          kind="ExternalInput").ap()
    wout_d = nc.dram_tensor("w_out_proc", [128, KC, D], bf16,
                            kind="ExternalInput").ap()
    iden_d = nc.dram_tensor("iden", [128, 128], f32,
                            kind="ExternalInput").ap()
    out_d = nc.dram_tensor("out", [T, D], f32, kind="ExternalOutput").ap()

    from contextlib import ExitStack

    with tile.TileContext(nc) as tc, ExitStack() as ctx:
        consts = ctx.enter_context(tc.tile_pool(name="consts", bufs=1))
        xp = ctx.enter_context(tc.tile_pool(name="xp", bufs=2))
        lnp = ctx.enter_context(tc.tile_pool(name="ln", bufs=4))
        xtp = ctx.enter_context(tc.tile_pool(name="xtp", bufs=2))
        rowp = ctx.enter_context(tc.tile_pool(name="rowp", bufs=2))
        bcp = ctx.enter_context(tc.tile_pool(name="bcp", bufs=2))
        scp = ctx.enter_context(tc.tile_pool(name="scp", bufs=2))
        hid2p = ctx.enter_context(tc.tile_pool(name="hid2", bufs=1))
        sgp = ctx.enter_context(tc.tile_pool(name="sg", bufs=2))
        hnp = ctx.enter_context(tc.tile_pool(name="hn", bufs=2))
        hbp = ctx.enter_context(tc.tile_pool(name="hb", bufs=2))
        accp = ctx.enter_context(tc.tile_pool(name="acc", bufs=2))
        outp = ctx.enter_context(tc.tile_pool(name="outp", bufs=2))
        xrp = ctx.enter_context(tc.tile_pool(name="xr", bufs=2))
        pst = ctx.enter_context(tc.tile_pool(name="pst", bufs=3, space="PSUM"))
        psm = ctx.enter_context(tc.tile_pool(name="psm", bufs=5, space="PSUM"))
        if True:
            win_sb = consts.tile([128, KC, 3 * DC * 128], bf16)
            for k in range(KC):
                nc.sync.dma_start(out=win_sb[:, k, :], in_=win_d[k])
            wout_sb = consts.tile([128, KC, D], bf16)
            for k in range(KC):
                nc.sync.dma_start(out=wout_sb[:, k, :], in_=wout_d[:, k, :])
            d_sb = consts.tile([128, KC, 3], f32)
            nc.sync.dma_start(out=d_sb, in_=d_d)
            cw_sb = consts.tile([128, KC, KCONV], f32)
            nc.sync.dma_start(out=cw_sb, in_=cw_d)
            cb_sb = consts.tile([128, KC], f32)
            nc.sync.dma_start(out=cb_sb, in_=cb_d)
            iden_sb = consts.tile([128, 128], f32)
            nc.sync.dma_start(out=iden_sb, in_=iden_d)
            eps_sb = consts.tile([128, 1], f32)
            nc.vector.memset(eps_sb, EPS)
            ones_sb = consts.tile([1, 128], bf16)
            nc.vector.memset(ones_sb, 1.0)
            state = consts.tile([128, KC, 3], bf16)
            nc.vector.memset(state, 0.0)

            for tb in range(NTB):
                t0 = tb * TB
                # ---- phase A: LN stats + broadcast-normalize xT ----
                xT = xtp.tile([128, DC, TB], bf16, name="xT")
                nc.sync.dma_start(out=xT, in_=xt_d[tb])
                mv4 = lnp.tile([128, 2 * NSUB], f32, name="mv4")
                for s in range(NSUB):
                    stats = lnp.tile([128, 2, 6], f32, name="stats")
                    for g in range(2):
                        xt = xp.tile([128, 512], f32, name="xt")
                        nc.sync.dma_start(
                            out=xt, in_=x_d[t0 + s * 128:t0 + (s + 1) * 128,
                                            g * 512:(g + 1) * 512])
                        nc.vector.bn_stats(out=stats[:, g, :], in_=xt)
                    # mean -> col s, var -> col 4+s
                    nc.vector.bn_aggr(out=mv4[:, s:s + 5:4], in_=stats)
                # rstd in-place on the var columns (tiny, full-width lanes)
                nc.scalar.activation(out=mv4[:, 4:8], in_=mv4[:, 4:8],
                                     func=AF.Sqrt, bias=eps_sb, scale=1.0)
                nc.vector.reciprocal(out=mv4[:, 4:8], in_=mv4[:, 4:8])
                # transpose [128, 8] -> [8, 128]: rows 0:4 mu_s, 4:8 rstd_s
                pt = pst.tile([128, TB], f32, name="bc")
                nc.tensor.transpose(pt[0:8, 0:128], mv4, iden_sb)
                mv8 = rowp.tile([8, 128], bf16, name="mv8")
                nc.scalar.copy(out=mv8, in_=pt[0:8, 0:128])
                mrow = rowp.tile([1, 2 * NSUB, 128], bf16, name="mrow")
                nc.sync.dma_start(out=mrow, in_=mv8)
                # rank-1 broadcasts: mu_b / rstd_b [128, TB]
                mu_ps = pst.tile([128, TB], f32, name="bc")
                nc.tensor.matmul(mu_ps, ones_sb, mrow[:, 0:NSUB, :],
                                 start=True, stop=True)
                rs_ps = pst.tile([128, TB], f32, name="bc")
                nc.tensor.matmul(rs_ps, ones_sb, mrow[:, NSUB:2 * NSUB, :],
                                 start=True, stop=True)
                mu_bs = bcp.tile([128, TB], bf16, name="mu_bs")
                nc.scalar.copy(out=mu_bs, in_=mu_ps)
                rs_bs = bcp.tile([128, TB], bf16, name="rs_bs")
                nc.scalar.copy(out=rs_bs, in_=rs_ps)
                for j in range(DC):
                    sc = scp.tile([128, TB], bf16, name="sc")
                    nc.vector.tensor_sub(out=sc, in0=xT[:, j, :], in1=mu_bs)
                    nc.vector.tensor_mul(out=xT[:, j, :], in0=sc, in1=rs_bs)

                # ---- phase B: in-proj + gates + conv per channel block ----
                hid2 = hid2p.tile([128, KC, TB], bf16, name="hid2")
                for k in range(KC):
                    ps3 = []
                    for t3 in range(3):
                        ps = psm.tile([128, TB], f32, name="mm")
                        for j in range(DC):
                            nc.tensor.matmul(
                                ps,
                                win_sb[:, k, (t3 * DC + j) * 128:
                                       (t3 * DC + j + 1) * 128],
                                xT[:, j, :],
                                start=(j == 0), stop=(j == DC - 1))
                        ps3.append(ps)
                    sigb = sgp.tile([128, TB], bf16, name="sigb")
                    nc.scalar.activation(out=sigb, in_=ps3[0], func=AF.Sigmoid,
                                         bias=d_sb[:, k, 0:1], scale=1.0)
                    sigc = sgp.tile([128, TB], bf16, name="sigc")
                    nc.scalar.activation(out=sigc, in_=ps3[1], func=AF.Sigmoid,
                                         bias=d_sb[:, k, 1:2], scale=1.0)
                    hnew = hnp.tile([128, TB], bf16, name="hnew")
                    nc.scalar.activation(out=hnew, in_=ps3[2],
                                         func=AF.Identity,
                                         bias=d_sb[:, k, 2:3], scale=1.0)
                    # hidb layout: [0] pad, [1:4] state, [4:TB+4] gated hidden
                    hidb = hbp.tile([128, TB + 4], bf16, name="hidb")
                    nc.vector.tensor_copy(out=hidb[:, 1:4], in_=state[:, k, :])
                    nc.vector.tensor_mul(out=hidb[:, 4:TB + 4], in0=hnew,
                                         in1=sigb)
                    nc.vector.tensor_copy(
                        out=state[:, k, :], in_=hidb[:, TB + 1:TB + 4])
                    a0 = accp.tile([128, TB], bf16, name="acc0")
                    a1 = accp.tile([128, TB], bf16, name="acc1")
                    # accumulate taps newest-to-oldest: start on the aligned
                    # [4:TB+4] window (4x tensor_scalar), then FMA down
                    nc.vector.tensor_scalar_mul(
                        out=a0, in0=hidb[:, 4:TB + 4],
                        scalar1=cw_sb[:, k, 3:4])
                    nc.vector.scalar_tensor_tensor(
                        out=a1, in0=hidb[:, 3:TB + 3],
                        scalar=cw_sb[:, k, 2:3], in1=a0,
                        op0=OP.mult, op1=OP.add)
                    nc.vector.scalar_tensor_tensor(
                        out=a0, in0=hidb[:, 2:TB + 2],
                        scalar=cw_sb[:, k, 1:2], in1=a1,
                        op0=OP.mult, op1=OP.add)
                    nc.vector.scalar_tensor_tensor(
                        out=a1, in0=hidb[:, 1:TB + 1],
                        scalar=cw_sb[:, k, 0:1], in1=a0,
                        op0=OP.mult, op1=OP.add)
                    nc.vector.scalar_tensor_tensor(
                        out=hid2[:, k, :], in0=a1, scalar=cb_sb[:, k:k + 1],
                        in1=sigc, op0=OP.add, op1=OP.mult)

                # ---- phase C: out-proj (j-outer, 2-bank groups) + residual ----
                for dh in range(2):
                    for sp in range(2):
                        ps0 = psm.tile([128, 512], f32, name="mm")
                        ps1 = psm.tile([128, 512], f32, name="mm")
                        s0, s1 = 2 * sp, 2 * sp + 1
                        for j in range(KC):
                            nc.tensor.matmul(
                                ps0,
                                hid2[:, j, s0 * 128:(s0 + 1) * 128],
                                wout_sb[:, j, dh * 512:(dh + 1) * 512],
                                start=(j == 0), stop=(j == KC - 1))
                            nc.tensor.matmul(
                                ps1,
                                hid2[:, j, s1 * 128:(s1 + 1) * 128],
                                wout_sb[:, j, dh * 512:(dh + 1) * 512],
                                start=(j == 0), stop=(j == KC - 1))
                        for s, ps in ((s0, ps0), (s1, ps1)):
                            xr = xrp.tile([128, 512], f32, name="xr")
                            nc.sync.dma_start(
                                out=xr,
                                in_=xr_d[t0 + s * 128:t0 + (s + 1) * 128,
                                         dh * 512:(dh + 1) * 512])
                            ot = outp.tile([128, 512], f32, name="ot")
                            nc.vector.tensor_add(out=ot, in0=ps, in1=xr)
                            nc.sync.dma_start(
                                out=out_d[t0 + s * 128:t0 + (s + 1) * 128,
                                          dh * 512:(dh + 1) * 512],
                                in_=ot)
    nc.compile()
    return nc


def _prep_inputs(x, ln_g, ln_b, w_in, b_in, conv_w, conv_b, w_out, b_out):
    import ml_dtypes

    bf16 = ml_dtypes.bfloat16
    x = np.asarray(x, np.float32)
    ln_g = np.asarray(ln_g, np.float32)
    ln_b = np.asarray(ln_b, np.float32)
    w_in = np.asarray(w_in, np.float32)
    b_in = np.asarray(b_in, np.float32)
    conv_w = np.asarray(conv_w, np.float32)
    conv_b = np.asarray(conv_b, np.float32)
    w_out = np.asarray(w_out, np.float32)
    b_out = np.asarray(b_out, np.float32)

    wg = w_in * ln_g[None, :]                       # [3H, D]
    dv = b_in + w_in @ ln_b                         # [3H]
    # [third, k, m, j, p] -> [k, p, third, j, m]
    wblk = wg.reshape(3, KC, 128, DC, 128)
    w_in_proc = np.ascontiguousarray(
        wblk.transpose(1, 4, 0, 3, 2)).reshape(KC, 128, 3 * DC * 128)
    w_in_proc = w_in_proc.astype(bf16)
    d_proc = np.ascontiguousarray(dv.reshape(3, KC, 128).transpose(2, 1, 0))
    cw_proc = np.ascontiguousarray(
        conv_w.reshape(KC, 128, KCONV).transpose(1, 0, 2))
    cb_proc = np.ascontiguousarray(conv_b.reshape(KC, 128).T)
    w_out_proc = np.ascontiguousarray(
        w_out.T.reshape(KC, 128, D).transpose(1, 0, 2)).astype(bf16)
    iden = np.eye(128, dtype=np.float32)

    shared = {
        "w_in_proc": w_in_proc, "d_proc": d_proc, "cw_proc": cw_proc,
        "cb_proc": cb_proc, "w_out_proc": w_out_proc, "iden": iden,
    }
    maps = []
    for b in range(B):
        xb = np.ascontiguousarray(x[b])
        # xt_proc[tb, p, j, t] = x[b, 512*tb + t, 128*j + p]
        xt_proc = np.ascontiguousarray(
            xb.T.reshape(DC, 128, NTB, TB).transpose(2, 1, 0, 3)).astype(bf16)
        xr_proc = xb + b_out[None, :]
        maps.append(dict(shared, x=xb, xt_proc=xt_proc, xr_proc=xr_proc))
    return maps


def run(inputs, trace=False):
    _install_ntff_hook()
    import concourse.bass_utils as bu

    bu.upload_artifacts = lambda d: "local://" + d
    if "nc" not in _CACHE:
        _CACHE["nc"] = _build_nc()
    nc = _CACHE["nc"]
    in_maps = _prep_inputs(**inputs)
    res = bu.run_bass_kernel_spmd(nc, in_maps, list(range(N_CORES)),
                                  trace=trace)
    out = np.stack([res.results[b]["out"] for b in range(B)])
    return out.astype(np.float32), res


def kernel(**inputs):
    out, _ = run(inputs, trace=False)
    return out


# revision 7
# speedup vs baseline: 1.6124x; 1.3532x over previous
"""Trainium2 Bass kernel for nn_GatedShortConvBlock.

Full inputs -> full output. Data-parallel over batch: batch b -> core b.
Per-core dataflow (T=4096 tokens, D=1024, H=2048, K=4):
  Weights resident in SBUF (loaded once): hidden-stream w_in + w_out in
  bf16, gate-stream w_in in fp8e4m3 (x64 scaled, compensated in the
  sigmoid scale) consumed with DoubleRow matmuls (K=256/instruction).
  LN stats on DVE (bn_stats over natural-layout x tiles) -> mean/rstd via
  tiny sqrt/recip + PE transpose -> rank-1 PE broadcast -> normalize the
  pre-transposed xT into bf16 (hidden mms) and fp8 (gate mms) copies ->
  in-proj matmuls (N=512) -> sigmoid gates + hidden bias on ACT (bf16
  out) -> depthwise causal conv on DVE bf16 FMAs -> out-proj bf16
  matmuls (j-outer 2-bank groups, pipelining behind the conv) ->
  +residual (host-fused x + b_out) -> out.
  Software pipelined: A(0); for tb: B(tb), A(tb+1), C(tb).
"""

import sys
import types

import numpy as np

B, T, D, H = 8, 4096, 1024, 2048
KCONV = 4
N_CORES = 8
TB = 512            # tokens per t-block
NTB = T // TB       # 8
NSUB = TB // 128    # 4
KC = H // 128       # 16 channel blocks
DC = D // 128       # 8 d-chunks
EPS = 1e-5
GSCALE = 64.0       # fp8 gate-weight pre-scale

_CACHE = {}


def _install_ntff_hook():
    if "antenv.axon_hooks" in sys.modules:
        return
    try:
        import trn_agent_boot.trn_boot as tb

        hook = tb._ntff_profile_via_ctypes("/opt/axon/libaxon_pjrt.so")
        mod = types.ModuleType("antenv.axon_hooks")
        mod.get_axon_ntff_profile_hook = lambda: hook
        mod.set_axon_ntff_profile_hook = lambda h: None
        sys.modules["antenv.axon_hooks"] = mod
    except Exception:
        pass


def _build_nc(dbg=False):
    import concourse.bass as bass  # noqa: F401
    import concourse.tile as tile
    from concourse import bacc, mybir

    f32 = mybir.dt.float32
    bf16 = mybir.dt.bfloat16
    fp8 = mybir.dt.float8e4
    AF = mybir.ActivationFunctionType
    OP = mybir.AluOpType
    DR = mybir.MatmulPerfMode.DoubleRow

    nc = bacc.Bacc("TRN2", target_bir_lowering=False, debug=False,
                   num_devices=N_CORES)
    x_d = nc.dram_tensor("x", [T, D], f32, kind="ExternalInput").ap()
    xr_d = nc.dram_tensor("xr_proc", [T, D], f32, kind="ExternalInput").ap()
    xt_d = nc.dram_tensor("xt_proc", [NTB, 128, DC, TB], bf16,
                          kind="ExternalInput").ap()
    winh_d = nc.dram_tensor("w_inh_proc", [KC, 128, DC * 128], bf16,
                            kind="ExternalInput").ap()
    wing_d = nc.dram_tensor("w_ing_proc", [KC, 128, 2 * DC * 128], fp8,
                            kind="ExternalInput").ap()
    d_d = nc.dram_tensor("d_proc", [128, KC, 3], f32,
                         kind="ExternalInput").ap()
    cw_d = nc.dram_tensor("cw_proc", [128, KC, KCONV], f32,
                          kind="ExternalInput").ap()
    cb_d = nc.dram_tensor("cb_proc", [128, KC], f32,
                          kind="ExternalInput").ap()
    wout_d = nc.dram_tensor("w_out_proc", [128, KC, D], bf16,
                            kind="ExternalInput").ap()
    iden_d = nc.dram_tensor("iden", [128, 128], f32,
                            kind="ExternalInput").ap()
    out_d = nc.dram_tensor("out", [T, D], f32, kind="ExternalOutput").ap()
    if dbg:
        dxt8_d = nc.dram_tensor("dbg_xt8", [128, DC, TB], mybir.dt.float8e4,
                                kind="ExternalOutput").ap()
        dxt_d = nc.dram_tensor("dbg_xt", [128, DC, TB], bf16,
                               kind="ExternalOutput").ap()
        dgb_d = nc.dram_tensor("dbg_gb", [128, TB], f32,
                               kind="ExternalOutput").ap()
        dsigb_d = nc.dram_tensor("dbg_sigb", [128, TB], bf16,
                                 kind="ExternalOutput").ap()
        dhid2_d = nc.dram_tensor("dbg_hid2", [128, KC, TB], bf16,
                                 kind="ExternalOutput").ap()

    from contextlib import ExitStack

    with tile.TileContext(nc) as tc, ExitStack() as ctx:
        consts = ctx.enter_context(tc.tile_pool(name="consts", bufs=1))
        xp = ctx.enter_context(tc.tile_pool(name="xp", bufs=2))
        lnp = ctx.enter_context(tc.tile_pool(name="ln", bufs=4))
        xtp = ctx.enter_context(tc.tile_pool(name="xtp", bufs=2))
        xt8p = ctx.enter_context(tc.tile_pool(name="xt8p", bufs=2))
        rowp = ctx.enter_context(tc.tile_pool(name="rowp", bufs=2))
        bcp = ctx.enter_context(tc.tile_pool(name="bcp", bufs=2))
        scp = ctx.enter_context(tc.tile_pool(name="scp", bufs=2))
        hid2p = ctx.enter_context(tc.tile_pool(name="hid2", bufs=1))
        sgp = ctx.enter_context(tc.tile_pool(name="sg", bufs=2))
        hnp = ctx.enter_context(tc.tile_pool(name="hn", bufs=2))
        hbp = ctx.enter_context(tc.tile_pool(name="hb", bufs=2))
        accp = ctx.enter_context(tc.tile_pool(name="acc", bufs=2))
        outp = ctx.enter_context(tc.tile_pool(name="outp", bufs=2))
        xrp = ctx.enter_context(tc.tile_pool(name="xr", bufs=2))
        pst = ctx.enter_context(tc.tile_pool(name="pst", bufs=3, space="PSUM"))
        psm = ctx.enter_context(tc.tile_pool(name="psm", bufs=5, space="PSUM"))
        if True:
            eps_sb = consts.tile([128, 1], f32)
            nc.vector.memset(eps_sb, EPS)
            ones_sb = consts.tile([1, 128], bf16)
            nc.vector.memset(ones_sb, 1.0)
            state = consts.tile([128, KC, 3], bf16)
            nc.vector.memset(state, 0.0)

            iden_sb = consts.tile([128, 128], f32)
            d_sb = consts.tile([128, KC, 3], f32)
            cw_sb = consts.tile([128, KC, KCONV], f32)
            cb_sb = consts.tile([128, KC], f32)
            winh_sb = consts.tile([128, KC, DC * 128], bf16)
            wing_sb = consts.tile([128, KC, 2, DC, 128], fp8)
            wout_sb = consts.tile([128, KC, D], bf16)

            abuf = {}

            def emit_consts_small():
                nc.sync.dma_start(out=iden_sb, in_=iden_d)
                nc.sync.dma_start(out=d_sb, in_=d_d)
                nc.sync.dma_start(out=cw_sb, in_=cw_d)
                nc.sync.dma_start(out=cb_sb, in_=cb_d)

            def emit_consts():
                for k in range(KC):
                    nc.sync.dma_start(out=winh_sb[:, k, :], in_=winh_d[k])
                    nc.sync.dma_start(out=wing_sb[:, k], in_=wing_d[k])
                for k in range(KC):
                    nc.sync.dma_start(out=wout_sb[:, k, :],
                                      in_=wout_d[:, k, :])

            def emit_A(tb):
                t0 = tb * TB
                xT = xtp.tile([128, DC, TB], bf16, name="xT")
                nc.sync.dma_start(out=xT, in_=xt_d[tb])
                xT8 = xt8p.tile([128, DC, TB], fp8, name="xT8")
                mv4 = lnp.tile([128, 2 * NSUB], f32, name="mv4")
                for s in range(NSUB):
                    stats = lnp.tile([128, 2, 6], f32, name="stats")
                    for g in range(2):
                        xt = xp.tile([128, 512], f32, name="xt")
                        nc.sync.dma_start(
                            out=xt, in_=x_d[t0 + s * 128:t0 + (s + 1) * 128,
                                            g * 512:(g + 1) * 512])
                        nc.vector.bn_stats(out=stats[:, g, :], in_=xt)
                    # mean -> col s, var -> col 4+s
                    nc.vector.bn_aggr(out=mv4[:, s:s + 5:4], in_=stats)
                # rstd in-place on the var columns (tiny, full-width lanes)
                nc.scalar.activation(out=mv4[:, 4:8], in_=mv4[:, 4:8],
                                     func=AF.Sqrt, bias=eps_sb, scale=1.0)
                nc.vector.reciprocal(out=mv4[:, 4:8], in_=mv4[:, 4:8])
                # transpose [128, 8] -> [8, 128]: rows 0:4 mu_s, 4:8 rstd_s
                pt = pst.tile([128, TB], f32, name="bc")
                nc.tensor.transpose(pt[0:8, 0:128], mv4, iden_sb)
                mv8 = rowp.tile([8, 128], bf16, name="mv8")
                nc.scalar.copy(out=mv8, in_=pt[0:8, 0:128])
                mrow = rowp.tile([1, 2 * NSUB, 128], bf16, name="mrow")
                nc.sync.dma_start(out=mrow, in_=mv8)
                # rank-1 broadcasts: mu_b / rstd_b [128, TB]
                mu_ps = pst.tile([128, TB], f32, name="bc")
                nc.tensor.matmul(mu_ps, ones_sb, mrow[:, 0:NSUB, :],
                                 start=True, stop=True)
                rs_ps = pst.tile([128, TB], f32, name="bc")
                nc.tensor.matmul(rs_ps, ones_sb, mrow[:, NSUB:2 * NSUB, :],
                                 start=True, stop=True)
                mu_bs = bcp.tile([128, TB], bf16, name="mu_bs")
                nc.scalar.copy(out=mu_bs, in_=mu_ps)
                rs_bs = bcp.tile([128, TB], bf16, name="rs_bs")
                nc.scalar.copy(out=rs_bs, in_=rs_ps)
                for j in range(DC):
                    sc = scp.tile([128, TB], bf16, name="sc")
                    nc.vector.tensor_sub(out=sc, in0=xT[:, j, :], in1=mu_bs)
                    nc.vector.tensor_mul(out=xT[:, j, :], in0=sc, in1=rs_bs)
                    nc.vector.tensor_mul(out=xT8[:, j, :], in0=sc, in1=rs_bs)
                if dbg and tb == 0:
                    nc.sync.dma_start(out=dxt8_d, in_=xT8)
                    nc.sync.dma_start(out=dxt_d, in_=xT)
                abuf[tb] = (xT, xT8)

            def emit_B(tb):
                xT, xT8 = abuf.pop(tb)
                hid2 = hid2p.tile([128, KC, TB], bf16, name="hid2")
                for k in range(KC):
                    ps_gb = psm.tile([128, TB], f32, name="mm")
                    for j2 in range(DC // 2):
                        nc.tensor.matmul(
                            ps_gb, wing_sb[:, k, 0, 2 * j2:2 * j2 + 2, :],
                            xT8[:, 2 * j2:2 * j2 + 2, :],
                            start=(j2 == 0), stop=(j2 == DC // 2 - 1),
                            perf_mode=DR)
                    ps_h = psm.tile([128, TB], f32, name="mm")
                    for j in range(DC):
                        nc.tensor.matmul(
                            ps_h, winh_sb[:, k, j * 128:(j + 1) * 128],
                            xT[:, j, :],
                            start=(j == 0), stop=(j == DC - 1))
                    ps_gc = psm.tile([128, TB], f32, name="mm")
                    for j2 in range(DC // 2):
                        nc.tensor.matmul(
                            ps_gc, wing_sb[:, k, 1, 2 * j2:2 * j2 + 2, :],
                            xT8[:, 2 * j2:2 * j2 + 2, :],
                            start=(j2 == 0), stop=(j2 == DC // 2 - 1),
                            perf_mode=DR)
                    if dbg and tb == 0 and k == 0:
                        dgb = sgp.tile([128, TB], f32, name="dgb")
                        nc.vector.tensor_copy(out=dgb, in_=ps_gb)
                        nc.sync.dma_start(out=dgb_d, in_=dgb)
                    sigb = sgp.tile([128, TB], bf16, name="sigb")
                    nc.scalar.activation(out=sigb, in_=ps_gb, func=AF.Sigmoid,
                                         bias=d_sb[:, k, 0:1],
                                         scale=1.0 / GSCALE)
                    sigc = sgp.tile([128, TB], bf16, name="sigc")
                    nc.scalar.activation(out=sigc, in_=ps_gc, func=AF.Sigmoid,
                                         bias=d_sb[:, k, 1:2],
                                         scale=1.0 / GSCALE)
                    hnew = hnp.tile([128, TB], bf16, name="hnew")
                    nc.scalar.activation(out=hnew, in_=ps_h,
                                         func=AF.Identity,
                                         bias=d_sb[:, k, 2:3], scale=1.0)
                    # hidb layout: [0] pad, [1:4] state, [4:TB+4] gated hidden
                    hidb = hbp.tile([128, TB + 4], bf16, name="hidb")
                    if dbg and tb == 0 and k == 0:
                        nc.sync.dma_start(out=dsigb_d, in_=sigb)
                    nc.vector.tensor_copy(out=hidb[:, 1:4], in_=state[:, k, :])
                    nc.vector.tensor_mul(out=hidb[:, 4:TB + 4], in0=hnew,
                                         in1=sigb)
                    nc.vector.tensor_copy(
                        out=state[:, k, :], in_=hidb[:, TB + 1:TB + 4])
                    a0 = accp.tile([128, TB], bf16, name="acc0")
                    a1 = accp.tile([128, TB], bf16, name="acc1")
                    # accumulate taps newest-to-oldest: start on the aligned
                    # [4:TB+4] window (4x tensor_scalar), then FMA down
                    nc.vector.tensor_scalar_mul(
                        out=a0, in0=hidb[:, 4:TB + 4],
                        scalar1=cw_sb[:, k, 3:4])
                    nc.vector.scalar_tensor_tensor(
                        out=a1, in0=hidb[:, 3:TB + 3],
                        scalar=cw_sb[:, k, 2:3], in1=a0,
                        op0=OP.mult, op1=OP.add)
                    nc.vector.scalar_tensor_tensor(
                        out=a0, in0=hidb[:, 2:TB + 2],
                        scalar=cw_sb[:, k, 1:2], in1=a1,
                        op0=OP.mult, op1=OP.add)
                    nc.vector.scalar_tensor_tensor(
                        out=a1, in0=hidb[:, 1:TB + 1],
                        scalar=cw_sb[:, k, 0:1], in1=a0,
                        op0=OP.mult, op1=OP.add)
                    nc.vector.scalar_tensor_tensor(
                        out=hid2[:, k, :], in0=a1, scalar=cb_sb[:, k:k + 1],
                        in1=sigc, op0=OP.add, op1=OP.mult)
                return hid2

            def emit_C(tb, hid2):
                if dbg and tb == 0:
                    nc.sync.dma_start(out=dhid2_d, in_=hid2)
                t0 = tb * TB
                for dh in range(2):
                    for sp in range(2):
                        ps0 = psm.tile([128, 512], f32, name="mm")
                        ps1 = psm.tile([128, 512], f32, name="mm")
                        s0, s1 = 2 * sp, 2 * sp + 1
                        for j in range(KC):
                            nc.tensor.matmul(
                                ps0,
                                hid2[:, j, s0 * 128:(s0 + 1) * 128],
                                wout_sb[:, j, dh * 512:(dh + 1) * 512],
                                start=(j == 0), stop=(j == KC - 1))
                            nc.tensor.matmul(
                                ps1,
                                hid2[:, j, s1 * 128:(s1 + 1) * 128],
                                wout_sb[:, j, dh * 512:(dh + 1) * 512],
                                start=(j == 0), stop=(j == KC - 1))
                        for s, ps in ((s0, ps0), (s1, ps1)):
                            xr = xrp.tile([128, 512], f32, name="xr")
                            nc.sync.dma_start(
                                out=xr,
                                in_=xr_d[t0 + s * 128:t0 + (s + 1) * 128,
                                         dh * 512:(dh + 1) * 512])
                            ot = outp.tile([128, 512], f32, name="ot")
                            nc.vector.tensor_add(out=ot, in0=ps, in1=xr)
                            nc.sync.dma_start(
                                out=out_d[t0 + s * 128:t0 + (s + 1) * 128,
                                          dh * 512:(dh + 1) * 512],
                                in_=ot)

            emit_consts_small()
            emit_A(0)
            emit_consts()
            for tb in range(NTB):
                hid2 = emit_B(tb)
                if tb + 1 < NTB:
                    emit_A(tb + 1)
                emit_C(tb, hid2)
    nc.compile()
    return nc


def _prep_inputs(x, ln_g, ln_b, w_in, b_in, conv_w, conv_b, w_out, b_out):
    import ml_dtypes

    bf16 = ml_dtypes.bfloat16
    fp8 = ml_dtypes.float8_e4m3fn
    x = np.asarray(x, np.float32)
    ln_g = np.asarray(ln_g, np.float32)
    ln_b = np.asarray(ln_b, np.float32)
    w_in = np.asarray(w_in, np.float32)
    b_in = np.asarray(b_in, np.float32)
    conv_w = np.asarray(conv_w, np.float32)
    conv_b = np.asarray(conv_b, np.float32)
    w_out = np.asarray(w_out, np.float32)
    b_out = np.asarray(b_out, np.float32)

    wg = w_in * ln_g[None, :]                       # [3H, D]
    dv = b_in + w_in @ ln_b                         # [3H]
    # [third, k, m, j, p] -> [k, p, third, j, m]
    wblk = wg.reshape(3, KC, 128, DC, 128)
    # hidden stream (third=2) in bf16: [k, p, j, m]
    w_inh_proc = np.ascontiguousarray(
        wblk[2].transpose(0, 3, 2, 1)).reshape(KC, 128, DC * 128).astype(bf16)
    # gate streams (third=0,1) in fp8, x64: [k, p, t3, j, m]
    w_ing_proc = np.ascontiguousarray(
        wblk[0:2].transpose(1, 4, 0, 3, 2)).reshape(KC, 128, 2 * DC * 128)
    w_ing_proc = np.clip(w_ing_proc * GSCALE, -448.0, 448.0).astype(fp8)
    d_proc = np.ascontiguousarray(dv.reshape(3, KC, 128).transpose(2, 1, 0))
    cw_proc = np.ascontiguousarray(
        conv_w.reshape(KC, 128, KCONV).transpose(1, 0, 2))
    cb_proc = np.ascontiguousarray(conv_b.reshape(KC, 128).T)
    w_out_proc = np.ascontiguousarray(
        w_out.T.reshape(KC, 128, D).transpose(1, 0, 2)).astype(bf16)
    iden = np.eye(128, dtype=np.float32)

    shared = {
        "w_inh_proc": w_inh_proc, "w_ing_proc": w_ing_proc, "d_proc": d_proc,
        "cw_proc": cw_proc, "cb_proc": cb_proc, "w_out_proc": w_out_proc,
        "iden": iden,
    }
    maps = []
    for b in range(B):
        xb = np.ascontiguousarray(x[b])
        # xt_proc[tb, p, j, t] = x[b, 512*tb + t, 128*j + p]
        xt_proc = np.ascontiguousarray(
            xb.T.reshape(DC, 128, NTB, TB).transpose(2, 1, 0, 3)).astype(bf16)
        xr_proc = xb + b_out[None, :]
        maps.append(dict(shared, x=xb, xt_proc=xt_proc, xr_proc=xr_proc))
    return maps


def run(inputs, trace=False):
    _install_ntff_hook()
    import concourse.bass_utils as bu

    bu.upload_artifacts = lambda d: "local://" + d
    if "nc" not in _CACHE:
        _CACHE["nc"] = _build_nc()
    nc = _CACHE["nc"]
    in_maps = _prep_inputs(**inputs)
    res = bu.run_bass_kernel_spmd(nc, in_maps, list(range(N_CORES)),
                                  trace=trace)
    out = np.stack([res.results[b]["out"] for b in range(B)])
    return out.astype(np.float32), res


def kernel(**inputs):
    out, _ = run(inputs, trace=False)
    return out
